# revision 22
# baseline (speedup 1.0000x reference)
"""Trainium2 Bass kernel for nn_MultiHeadLocalAttention (band-limited attention).

Math: scores are multiplied by a band-count matrix C that is zero outside
|q-k|<=4, then a FULL-row softmax is applied.  Out-of-band entries contribute
exp(0)=1, so with E = exp(C*S) over a 136-wide halo band per 128-query tile:

  out[q]   = (sum_A E[k,q] vh[k] + sum_B (E[k,q]-1) vh[k] + Vcomp_j) / denom[q]
  denom[q] = [same with vh's 65th ones-column] + 1920

where Vcomp_j = (sum_all vh) - (sum_{k in A_j} vh) is computed host-side in
fp64 per q-tile j.  This makes the O(seq^2) attention O(seq*band), exact to
fp rounding, with no "-1" elementwise pass on the big A tile.

Sharding: batch*seq rows split across 8 cores (512 rows each); each core
computes all 8 heads for its rows.  k/v inputs carry a +-4 halo.  All matmul
operands are bf16; PSUM accumulation is fp32.

Perf structure: projections use full M=128 stationary tiles (head-pairs
packed 64+64 in the partition dim; score matmuls address the PE quadrants
via base-partition 0/64).  The PE instruction stream interleaves v-proj,
scores/numer, transpose and out-proj so the tensor engine stays dense (keeps
the 2.4GHz p-state).  DMA loads are ordered by first use on two HWDGE
queues; output tiles stream out as they finish.
"""

import math
import sys
from contextlib import ExitStack

import numpy as np

sys.path.insert(0, "/opt/trn_rl_repo")

import ml_dtypes

import concourse.bass as bass
import concourse.tile as tile
from concourse import bacc, mybir
from concourse.bass_utils import run_bass_kernel_spmd

F32 = mybir.dt.float32
BF16 = mybir.dt.bfloat16
NP_BF16 = ml_dtypes.bfloat16
SEQ, DM, H, DK = 2048, 512, 8, 64
ROWS = 512          # query rows per core
HALO = ROWS + 8     # padded k/v halo columns per core
J = 4               # 128-row query tiles per core
WBAND = 2

MULT = mybir.AluOpType.mult
EXP = mybir.ActivationFunctionType.Exp
COPYF = mybir.ActivationFunctionType.Copy

# head order inside the grouped pipelines: evens (PE row-quadrant 0) then
# odds (quadrant 64) -- minimizes PE tile-position switches per j.
HE = [0, 2, 4, 6]
HO = [1, 3, 5, 7]


# ----------------------------------------------------------------------------
# host-side helpers
# ----------------------------------------------------------------------------

def _band_count(seq=SEQ, window=WBAND):
    i = np.arange(seq)
    lo = np.clip(i - window, 0, None)
    hi = np.clip(i + window, None, seq - 1)
    lo = np.where(i == 1, 0, lo)
    hi = np.where(i == 1, window + 1, hi)
    lo = np.where(i == seq - 2, seq - window - 2, lo)
    hi = np.where(i == seq - 2, seq - 1, hi)
    a = np.arange(seq)[None, :]
    M = ((a >= lo[:, None]) & (a <= hi[:, None])).astype(np.float32)
    return M.T @ M


def _c_tiles(R0, C):
    """CA [J,128,136] (wide: cols 0:8 = prev tile's last-8 q), CB [J,8,128]."""
    CA = np.zeros((J, 128, 136), np.float32)
    CB = np.zeros((J, 8, 128), np.float32)
    for j in range(J):
        qg = R0 + 128 * j + np.arange(-8, 128)
        kgA = R0 - 4 + 128 * j + np.arange(128)
        kgB = R0 - 4 + 128 * j + 128 + np.arange(8)
        mA = (kgA >= 0) & (kgA < SEQ)
        mB = (kgB >= 0) & (kgB < SEQ)
        mQ = (qg >= 0) & (qg < SEQ)
        CA[j][np.ix_(mA, mQ)] = C[np.ix_(kgA[mA], qg[mQ])]
        CB[j][mB, :] = C[np.ix_(kgB[mB], R0 + 128 * j + np.arange(128))]
    return CA, CB


# ----------------------------------------------------------------------------
# device program
# ----------------------------------------------------------------------------

def _bcast(ap_, ins_dim, n):
    """Insert a [0, n] broadcast free dim at position ins_dim of ap_.ap."""
    dims = [list(d) for d in ap_.ap]
    dims.insert(ins_dim, [0, n])
    return bass.AP(tensor=ap_.tensor, offset=ap_.offset, ap=dims)


def _restride(ap_, new_free):
    """Replace the free dims of ap_ (keep partition dim + offset)."""
    dims = [list(ap_.ap[0])] + [list(d) for d in new_free]
    return bass.AP(tensor=ap_.tensor, offset=ap_.offset, ap=dims)


def _build_program(with_bias, debug=False):
    nc = bacc.Bacc()
    E_IN = dict(kind="ExternalInput")
    qT_d = nc.dram_tensor("qT", [DM, ROWS], BF16, **E_IN)
    kT_d = nc.dram_tensor("kT", [DM, HALO], BF16, **E_IN)
    vT_d = nc.dram_tensor("vT", [DM, HALO], BF16, **E_IN)
    Wq_d = nc.dram_tensor("Wq", [DM, DM], BF16, **E_IN)
    Wk_d = nc.dram_tensor("Wk", [DM, DM], BF16, **E_IN)
    Wv_d = nc.dram_tensor("Wv", [DM, DM], BF16, **E_IN)
    Wo_d = nc.dram_tensor("Wo", [DM, DM], BF16, **E_IN)
    CA_d = nc.dram_tensor("CA", [J, 128, 136], BF16, **E_IN)
    CBs_d = nc.dram_tensor("CBs", [J, 8, 8], BF16, **E_IN)
    vsj_d = nc.dram_tensor("vsj", [1, 5 * H * 65], BF16, **E_IN)
    ident_d = nc.dram_tensor("ident", [128, 128], BF16, **E_IN)
    if with_bias:
        bq_d = nc.dram_tensor("bq", [1, DM], BF16, **E_IN)
        bk_d = nc.dram_tensor("bk", [1, DM], BF16, **E_IN)
        bv_d = nc.dram_tensor("bv", [1, DM], BF16, **E_IN)
    out_d = nc.dram_tensor("out", [ROWS, DM], BF16, kind="ExternalOutput")
    if debug:
        dbg = {nm: nc.dram_tensor(f"dbg_{nm}", [128, sz], BF16,
                                  kind="ExternalOutput")
               for nm, sz in [("qhT", 4 * ROWS), ("khT", 4 * HALO),
                              ("vh", 5 * H * 65),
                              ("eB", H * 128), ("cd", J * DM),
                              ("cT", 4 * ROWS)]}

    with tile.TileContext(nc) as tc, ExitStack() as ctx:
        sing = ctx.enter_context(tc.tile_pool(name="sing", bufs=1))
        work = ctx.enter_context(tc.tile_pool(name="work", bufs=2))
        worke = ctx.enter_context(tc.tile_pool(name="worke", bufs=4))
        psa = ctx.enter_context(tc.tile_pool(name="psa", bufs=4, space="PSUM"))
        ppn = ctx.enter_context(tc.tile_pool(name="ppn", bufs=2, space="PSUM"))

        # ---- persistent SBUF tiles -------------------------------------
        sb_vT = sing.tile([128, 4, HALO], BF16)
        sb_qT = sing.tile([128, 4, ROWS], BF16)
        sb_kT = sing.tile([128, 4, HALO], BF16)
        sb_Wv = sing.tile([128, 4, DM], BF16)
        sb_Wq = sing.tile([128, 4, DM], BF16)
        sb_Wk = sing.tile([128, 4, DM], BF16)
        sb_Wo = sing.tile([128, 4, DM], BF16)
        sb_CA = sing.tile([128, J, 136], BF16)
        sb_CBs = sing.tile([8, J, 8], BF16)

        sb_id = sing.tile([128, 128], BF16)
        sb_ones = sing.tile([1, ROWS], BF16)
        sb_eB = sing.tile([33, H, 128], BF16)
        sb_e4 = sing.tile([8, H, 8], BF16)

        sb_qhT = sing.tile([128, 4, ROWS], BF16)     # [dout(128hp+p), hp, r]
        sb_khT = sing.tile([128, 4, HALO], BF16)
        sb_vh = sing.tile([128, 5, H, 65], BF16)     # shifted row tiles
        sb_vb = sing.tile([33, 5, H, 65], BF16)      # B-halo rows + comp row@32
        sb_cd = sing.tile([128, J, DM], BF16)        # concat/attn out
        sb_cT = sing.tile([128, 4, ROWS], BF16)      # concat transposed

        # ---- DMA loads, ordered by first use ---------------------------
        # sync (SP) queue: activations; vector (DVE) queue: weights.
        nc.sync.dma_start(sb_vT[:, :, 0:264],
                          vT_d[:, 0:264].rearrange("(kc p) r -> p kc r", p=128))
        nc.scalar.dma_start(sb_Wv, Wv_d[:].rearrange("(kc p) n -> p kc n", p=128))
        nc.sync.dma_start(sb_qT, qT_d[:].rearrange("(kc p) r -> p kc r", p=128))
        nc.scalar.dma_start(sb_Wq, Wq_d[:].rearrange("(kc p) n -> p kc n", p=128))
        nc.sync.dma_start(sb_vT[:, :, 264:HALO],
                          vT_d[:, 264:HALO].rearrange("(kc p) r -> p kc r", p=128))
        nc.scalar.dma_start(sb_Wk, Wk_d[:].rearrange("(kc p) n -> p kc n", p=128))
        nc.sync.dma_start(sb_kT, kT_d[:].rearrange("(kc p) r -> p kc r", p=128))
        nc.scalar.dma_start(sb_CA, CA_d[:].rearrange("j p q -> p j q"))
        nc.scalar.dma_start(sb_Wo, Wo_d[:].rearrange("(kc p) n -> p kc n", p=128))
        nc.gpsimd.memset(sb_ones, 1.0)
        nc.gpsimd.memset(sb_eB, 0.0)
        nc.gpsimd.memset(sb_eB[32:33, :, :], 1.0)
        nc.gpsimd.memset(sb_vh[:, :, :, 64:65], 1.0)
        nc.gpsimd.dma_start(sb_CBs, CBs_d[:].rearrange("j p q -> p j q"))
        nc.gpsimd.memset(sb_vb, 0.0)
        nc.gpsimd.dma_start(sb_vb[32:33, :, :, :],
                            vsj_d[:].rearrange("p (t h d) -> p t h d", t=5, h=H))
        nc.gpsimd.dma_start(sb_id, ident_d[:])
        if with_bias:
            sb_bq = sing.tile([1, DM], BF16)
            nc.gpsimd.dma_start(sb_bq, bq_d[:])
            sb_bk = sing.tile([1, DM], BF16)
            nc.gpsimd.dma_start(sb_bk, bk_d[:])
            sb_bv = sing.tile([1, DM], BF16)
            nc.gpsimd.dma_start(sb_bv, bv_d[:])


        # evac engines round-robin
        ACTC = lambda o, i: nc.scalar.copy(o, i)
        DVEC = lambda o, i: nc.vector.tensor_copy(o, i)
        POOLC = lambda o, i: nc.gpsimd.tensor_copy(o, i)

        def vproj(rt, ps, eng):
            nr = 128 if rt < 4 else 8
            for kc in range(4):
                nc.tensor.matmul(ps[0:nr, :],
                                 sb_vT[:, kc, 128 * rt:128 * rt + nr],
                                 sb_Wv[:, kc, :], start=(kc == 0),
                                 stop=(kc == 3 and not with_bias))
            if with_bias:
                nc.tensor.matmul(ps[0:nr, :], sb_ones[0:1, 0:nr], sb_bv,
                                 start=False, stop=True)
            eng(sb_vh[0:nr, rt, :, 0:64],
                ps[0:nr, :].rearrange("p (h d) -> p h d", h=H))
            nc.gpsimd.tensor_copy(sb_vb[0:8, rt, :, :], sb_vh[0:8, rt, :, :])

        def proj_qk(hp, W, x, xcols, dst, bias, ps, eng):
            c0, c1 = xcols
            for kc in range(4):
                nc.tensor.matmul(ps[:, 0:c1 - c0],
                                 W[:, kc, 128 * hp:128 * hp + 128],
                                 x[:, kc, c0:c1], start=(kc == 0),
                                 stop=(kc == 3 and not with_bias))
            if with_bias:
                nc.tensor.matmul(ps[:, 0:c1 - c0],
                                 bias[0:1, 128 * hp:128 * hp + 128],
                                 sb_ones[0:1, 0:c1 - c0], start=False, stop=True)
            eng(dst[:, hp, c0:c1], ps[:, 0:c1 - c0])

        def scores(j, grp):
            # wide score tiles [kposA_j, q in [128j-8, 128j+128)]: the first
            # 8 q-columns are the B-piece of tile j-1 (kpos rows 0:8 there).
            hs = HE if grp == 0 else HO
            sas = []
            for t in range(2):
                sa = psa.tile([128, 2, 136], F32, tag="sa",
                              name=f"sa{j}{grp}{t}")
                if j == 0:
                    nc.vector.memset(sa[:, :, 0:8], 0.0)
                for i in range(2):
                    h = hs[2 * t + i]
                    lo = 64 * (h % 2)
                    if j == 0:
                        nc.tensor.matmul(
                            sa[:, i, 8:136],
                            sb_khT[lo:lo + 64, h // 2, 0:128],
                            sb_qhT[lo:lo + 64, h // 2, 0:128],
                            start=True, stop=True)
                    else:
                        nc.tensor.matmul(
                            sa[:, i, :],
                            sb_khT[lo:lo + 64, h // 2, 128 * j:128 * j + 128],
                            sb_qhT[lo:lo + 64, h // 2,
                                   128 * j - 8:128 * j + 128],
                            start=True, stop=True)
                sas.append(sa)
            return sas

        def psb4_all():
            # virtual tile-4 B piece for j=3 (kpos 512:520 x last 8 q),
            # both head groups; raw exp staged in sb_e4 (extracted later).
            pb = ppn.tile([8, 8, 8], F32, tag="pn", name="pb4")
            for g, hs in ((0, HE), (1, HO)):
                for i, h in enumerate(hs):
                    lo = 64 * (h % 2)
                    nc.tensor.matmul(
                        pb[0:8, 4 * g + i, :],
                        sb_khT[lo:lo + 64, h // 2, 512:520],
                        sb_qhT[lo:lo + 64, h // 2, 504:512],
                        start=True, stop=True)
            cb = sb_CBs[:, 3, :]
            nc.vector.tensor_mul(pb[0:8], pb[0:8], _bcast(cb, 1, H))
            nc.scalar.activation(sb_e4[0:8], pb[0:8], EXP)

        def psb4_ew(grp):
            ebv = sb_eB[0:8, 4 * grp:4 * grp + 4, 120:128]
            nc.gpsimd.tensor_scalar_add(
                ebv, sb_e4[0:8, 4 * grp:4 * grp + 4, :], -1.0)

        def band_ew(j, grp, sas, eA):
            """elementwise band pipeline for one head-group: C*S then exp."""
            ca = sb_CA[:, j, :]
            for t, sa in enumerate(sas):
                nc.vector.tensor_mul(sa[:], sa[:], _bcast(ca, 1, 2))
                nc.scalar.activation(eA[:, 2 * t:2 * t + 2, :], sa[:], EXP)

        def extract_eB(grp, eA_next):
            # B-piece of tile j-1 = (exp values in cols 0:8, rows 0:8 of the
            # next tile's wide eA) - 1, scattered into the zeroed eB band.
            ebv = sb_eB[0:8, 4 * grp:4 * grp + 4, 120:128]
            nc.gpsimd.tensor_scalar_add(ebv, eA_next[0:8, :, 0:8], -1.0)

        def numer(j, grp, eA):
            # psum accumulation groups must be sequential per bank: each
            # head opens (start), accumulates, and closes (stop) before the
            # next head's start, or the bank contents get reset.
            hs = HE if grp == 0 else HO
            pn = ppn.tile([128, 4, 65], F32, tag="pn", name=f"pn{j}{grp}")
            for i, h in enumerate(hs):
                nc.tensor.matmul(pn[:, i, :], eA[:, i, 8:136],
                                 sb_vh[:, j, h, :], start=True, stop=False)
                # 9-row B piece: row 8 is (eB=1) x (vb=host comp vector),
                # folding the Vcomp/denominator constant into this matmul.
                nc.tensor.matmul(pn[:, i, :], sb_eB[0:33, 4 * grp + i, :],
                                 sb_vb[0:33, j + 1, h, :],
                                 start=False, stop=True)
            return pn

        def divide(j, grp, pn):
            rf = worke.tile([128, 4], F32, tag="rf")
            nc.vector.reciprocal(rf, pn[:, :, 64])
            cd_g = _restride(sb_cd[:, j, 64 * grp:DM], [[128, 4], [1, 64]])
            nc.scalar.copy(cd_g, pn[:, :, 0:64])
            nc.gpsimd.tensor_mul(cd_g, cd_g, _bcast(rf, 2, 64))

        def transpose_rc(rc, pt):
            t = pt.tile([128, 4, 128], BF16, tag="t", name=f"t{rc}")
            for dc in range(4):
                nc.tensor.transpose(t[:, dc, :],
                                    sb_cd[:, rc, 128 * dc:128 * dc + 128],
                                    sb_id)
            (DVEC if rc % 2 == 0 else ACTC)(
                sb_cT[:, :, 128 * rc:128 * rc + 128], t)

        def oproj_rc(rc, pf):
            pfo = pf.tile([128, DM], F32, tag="f", name=f"pf{rc}")
            for dc in range(4):
                nc.tensor.matmul(pfo, sb_cT[:, dc, 128 * rc:128 * rc + 128],
                                 sb_Wo[:, dc, :], start=(dc == 0), stop=(dc == 3))
            fo = work.tile([128, DM], BF16, tag="fo")
            (ACTC if rc % 2 == 0 else DVEC)(fo, pfo)
            nc.sync.dma_start(out_d[128 * rc:128 * rc + 128, :], fo)

        # ================= phase A: projections =========================
        with tc.tile_pool(name="ppj", bufs=2, space="PSUM") as ppj:
            for rt, eng in ((0, ACTC), (1, DVEC)):
                vproj(rt, ppj.tile([128, DM], F32, tag="pj", name=f"pv{rt}"), eng)
            for hp, eng in ((0, ACTC), (1, DVEC), (2, ACTC), (3, DVEC)):
                proj_qk(hp, sb_Wq, sb_qT, (0, ROWS), sb_qhT,
                        sb_bq if with_bias else None,
                        ppj.tile([128, DM], F32, tag="pj", name=f"pq{hp}"), eng)
            for hp, eng in ((0, DVEC), (1, ACTC), (2, DVEC), (3, ACTC)):
                proj_qk(hp, sb_Wk, sb_kT, (0, 512), sb_khT,
                        sb_bk if with_bias else None,
                        ppj.tile([128, DM], F32, tag="pj", name=f"pk{hp}"), eng)
                proj_qk(hp, sb_Wk, sb_kT, (512, HALO), sb_khT,
                        sb_bk if with_bias else None,
                        ppj.tile([128, DM], F32, tag="pj", name=f"pkt{hp}"), DVEC)
            UNITS = [(jj, gg) for jj in range(J) for gg in range(2)]
            eAs = {}
            for u in (0, 1, 2):
                ju, gu = UNITS[u]
                sas = scores(ju, gu)
                eAs[u] = worke.tile([128, 4, 136], BF16, tag="eA",
                                    name=f"eA{u}")
                band_ew(ju, gu, sas, eAs[u])
            for rt, eng in ((2, DVEC), (3, ACTC)):
                vproj(rt, ppj.tile([128, DM], F32, tag="pj", name=f"pv{rt}"), eng)

        # ================= phase B: attention + output ==================
        with tc.tile_pool(name="pt", bufs=1, space="PSUM") as pt, \
             tc.tile_pool(name="pf", bufs=1, space="PSUM") as pf:
            for u in range(8):
                ju, gu = UNITS[u]
                if u + 3 < 8:
                    j3, g3 = UNITS[u + 3]
                    sas = scores(j3, g3)
                    eAs[u + 3] = worke.tile([128, 4, 136], BF16, tag="eA",
                                            name=f"eA{u + 3}")
                    band_ew(j3, g3, sas, eAs[u + 3])
                if u == 3:
                    psb4_all()
                if u <= 5:
                    extract_eB(gu, eAs[u + 2])
                else:
                    psb4_ew(gu)
                pn = numer(ju, gu, eAs.pop(u))
                divide(ju, gu, pn)
                if u == 3:
                    vproj(4, pt.tile([128, DM], F32, tag="t", name="pv4"),
                          DVEC)
                if gu == 1:
                    transpose_rc(ju, pt)
                    oproj_rc(ju, pf)
            if debug:
                nc.gpsimd.dma_start(dbg["qhT"][:],
                                    sb_qhT[:].rearrange("p a b -> p (a b)"))
                nc.gpsimd.dma_start(dbg["khT"][:],
                                    sb_khT[:].rearrange("p a b -> p (a b)"))
                nc.gpsimd.dma_start(dbg["vh"][:],
                                    sb_vh[:].rearrange("p a b c -> p (a b c)"))
                nc.gpsimd.dma_start(dbg["eB"][0:8],
                                    sb_eB[:].rearrange("p a b -> p (a b)"))
                nc.gpsimd.dma_start(dbg["cd"][:],
                                    sb_cd[:].rearrange("p a b -> p (a b)"))
                nc.gpsimd.dma_start(dbg["cT"][:],
                                    sb_cT[:].rearrange("p a b -> p (a b)"))

    if not nc.is_finalized():
        nc.finalize()
    return nc


_PROG_CACHE = {}


def _get_program(with_bias):
    import os
    dbg = bool(int(os.environ.get("BASS_KERNEL_DEBUG", "0")))
    key = (bool(with_bias), dbg)
    if key not in _PROG_CACHE:
        _PROG_CACHE[key] = _build_program(with_bias, debug=dbg)
    return _PROG_CACHE[key]


# ----------------------------------------------------------------------------
# entry point
# ----------------------------------------------------------------------------

def prep_in_maps(q, k, v, Wq, bq, Wk, bk, Wv, bv, Wo, bo, **_unused):
    """Builds per-core input maps + the traced program; returns (in_maps, nc)."""
    q = np.asarray(q, np.float32)
    k = np.asarray(k, np.float32)
    v = np.asarray(v, np.float32)
    Wq_b = np.ascontiguousarray(Wq, np.float32).astype(NP_BF16)
    Wk_b = np.ascontiguousarray(Wk, np.float32).astype(NP_BF16)
    Wv_b = np.ascontiguousarray(Wv, np.float32).astype(NP_BF16)
    Wo_b = np.ascontiguousarray(Wo, np.float32).astype(NP_BF16)
    bq = np.asarray(bq, np.float32).reshape(-1)
    bk = np.asarray(bk, np.float32).reshape(-1)
    bv = np.asarray(bv, np.float32).reshape(-1)
    with_bias = bool(np.any(bq) or np.any(bk) or np.any(bv))
    nc = _get_program(with_bias)

    C = _band_count() / np.float32(math.sqrt(DK))
    ident = np.eye(128, dtype=NP_BF16)
    Wv64 = Wv_b.astype(np.float64)
    bv64 = bv.astype(np.float64)

    in_maps = []
    for c in range(8):
        b, R0 = c // 4, ROWS * (c % 4)
        qT = np.ascontiguousarray(q[b, R0:R0 + ROWS, :].T).astype(NP_BF16)
        kT = np.zeros((DM, HALO), NP_BF16)
        vT = np.zeros((DM, HALO), NP_BF16)
        g0 = R0 - 4
        s0, s1 = max(g0, 0), min(R0 + ROWS + 4, SEQ)
        kT[:, s0 - g0:s1 - g0] = k[b, s0:s1, :].T.astype(NP_BF16)
        vT[:, s0 - g0:s1 - g0] = v[b, s0:s1, :].T.astype(NP_BF16)
        CA, CB = _c_tiles(R0, C)
        CBs = np.ascontiguousarray(CB[:, :, 120:128]).astype(NP_BF16)
        assert not CB[:, :, :120].any()
        # per-j compensation: Vcomp_j = sum_all vh - sum_{A_j valid} vh;
        # denominator constant is 2048 - 128 = 1920 exactly (every A row,
        # valid or padded, contributes +1 via the ones-column on device).
        vsum = v[b].sum(axis=0, dtype=np.float64)
        VsumW = vsum @ Wv64 + float(SEQ) * bv64
        vsj = np.zeros((5, H, 65), np.float64)
        for j in range(J):
            lo, hi = max(g0 + 128 * j, 0), min(g0 + 128 * j + 128, SEQ)
            nvalid = hi - lo
            svA = v[b, lo:hi].sum(axis=0, dtype=np.float64) @ Wv64 \
                + float(nvalid) * bv64
            vsj[j + 1, :, 0:64] = (VsumW - svA).reshape(H, DK)
            vsj[j + 1, :, 64] = float(SEQ - 128)
        m = {"qT": qT, "kT": kT, "vT": vT, "Wq": Wq_b, "Wk": Wk_b, "Wv": Wv_b,
             "Wo": Wo_b, "CA": CA.astype(NP_BF16), "CBs": CBs,
             "vsj": vsj.reshape(1, 5 * H * 65).astype(NP_BF16),
             "ident": ident}
        if with_bias:
            m["bq"] = bq[None, :].astype(NP_BF16)
            m["bk"] = bk[None, :].astype(NP_BF16)
            m["bv"] = bv[None, :].astype(NP_BF16)
        in_maps.append(m)
    return in_maps, nc


def kernel(q, k, v, Wq, bq, Wk, bk, Wv, bv, Wo, bo, **_unused):
    bo = np.asarray(bo, np.float32).reshape(-1)
    in_maps, nc = prep_in_maps(q, k, v, Wq, bq, Wk, bk, Wv, bv, Wo, bo)
    res = run_bass_kernel_spmd(nc, in_maps, core_ids=list(range(8)))
    out = np.empty((2, SEQ, DM), np.float32)
    for c in range(8):
        b, R0 = c // 4, ROWS * (c % 4)
        out[b, R0:R0 + ROWS, :] = res.results[c]["out"].astype(np.float32)
    if np.any(bo):
        out += bo
    return out


if __name__ == "__main__":
    rng = np.random.default_rng(0)
    s = 1.0 / math.sqrt(DM)
    inp = dict(
        q=rng.standard_normal((2, SEQ, DM)).astype(np.float32),
        k=rng.standard_normal((2, SEQ, DM)).astype(np.float32),
        v=rng.standard_normal((2, SEQ, DM)).astype(np.float32),
        Wq=(rng.standard_normal((DM, DM)) * s).astype(np.float32),
        bq=np.zeros(DM, np.float32),
        Wk=(rng.standard_normal((DM, DM)) * s).astype(np.float32),
        bk=np.zeros(DM, np.float32),
        Wv=(rng.standard_normal((DM, DM)) * s).astype(np.float32),
        bv=np.zeros(DM, np.float32),
        Wo=(rng.standard_normal((DM, DM)) * s).astype(np.float32),
        bo=np.zeros(DM, np.float32),
    )
    out = kernel(**inp)
    print("kernel ran, out shape", out.shape, "mean", np.abs(out).mean())


# revision 23
# speedup vs baseline: 1.0234x; 1.0234x over previous
"""Trainium2 Bass kernel for nn_MultiHeadLocalAttention (band-limited attention).

Math: scores are multiplied by a band-count matrix C that is zero outside
|q-k|<=4, then a FULL-row softmax is applied.  Out-of-band entries contribute
exp(0)=1, so with E = exp(C*S) over a 136-wide halo band per 128-query tile:

  out[q]   = (sum_A E[k,q] vh[k] + sum_B (E[k,q]-1) vh[k] + Vcomp_j) / denom[q]
  denom[q] = [same with vh's 65th ones-column] + 1920

where Vcomp_j = (sum_all vh) - (sum_{k in A_j} vh) is computed host-side in
fp64 per q-tile j.  This makes the O(seq^2) attention O(seq*band), exact to
fp rounding, with no "-1" elementwise pass on the big A tile.

Sharding: batch*seq rows split across 8 cores (512 rows each); each core
computes all 8 heads for its rows.  k/v inputs carry a +-4 halo.  All matmul
operands are bf16; PSUM accumulation is fp32.

Perf structure: projections use full M=128 stationary tiles (head-pairs
packed 64+64 in the partition dim; score matmuls address the PE quadrants
via base-partition 0/64).  The PE instruction stream interleaves v-proj,
scores/numer, transpose and out-proj so the tensor engine stays dense (keeps
the 2.4GHz p-state).  DMA loads are ordered by first use on two HWDGE
queues; output tiles stream out as they finish.
"""

import math
import sys
from contextlib import ExitStack

import numpy as np

sys.path.insert(0, "/opt/trn_rl_repo")

import ml_dtypes

import concourse.bass as bass
import concourse.tile as tile
from concourse import bacc, mybir
from concourse.bass_utils import run_bass_kernel_spmd

F32 = mybir.dt.float32
BF16 = mybir.dt.bfloat16
NP_BF16 = ml_dtypes.bfloat16
SEQ, DM, H, DK = 2048, 512, 8, 64
ROWS = 512          # query rows per core
HALO = ROWS + 8     # padded k/v halo columns per core
J = 4               # 128-row query tiles per core
WBAND = 2

MULT = mybir.AluOpType.mult
EXP = mybir.ActivationFunctionType.Exp
COPYF = mybir.ActivationFunctionType.Copy

# head order inside the grouped pipelines: evens (PE row-quadrant 0) then
# odds (quadrant 64) -- minimizes PE tile-position switches per j.
HE = [0, 2, 4, 6]
HO = [1, 3, 5, 7]


# ----------------------------------------------------------------------------
# host-side helpers
# ----------------------------------------------------------------------------

def _band_count(seq=SEQ, window=WBAND):
    i = np.arange(seq)
    lo = np.clip(i - window, 0, None)
    hi = np.clip(i + window, None, seq - 1)
    lo = np.where(i == 1, 0, lo)
    hi = np.where(i == 1, window + 1, hi)
    lo = np.where(i == seq - 2, seq - window - 2, lo)
    hi = np.where(i == seq - 2, seq - 1, hi)
    a = np.arange(seq)[None, :]
    M = ((a >= lo[:, None]) & (a <= hi[:, None])).astype(np.float32)
    return M.T @ M


def _c_tiles(R0, C):
    """CA [J,128,136] (wide: cols 0:8 = prev tile's last-8 q), CB [J,8,128]."""
    CA = np.zeros((J, 128, 136), np.float32)
    CB = np.zeros((J, 8, 128), np.float32)
    for j in range(J):
        qg = R0 + 128 * j + np.arange(-8, 128)
        kgA = R0 - 4 + 128 * j + np.arange(128)
        kgB = R0 - 4 + 128 * j + 128 + np.arange(8)
        mA = (kgA >= 0) & (kgA < SEQ)
        mB = (kgB >= 0) & (kgB < SEQ)
        mQ = (qg >= 0) & (qg < SEQ)
        CA[j][np.ix_(mA, mQ)] = C[np.ix_(kgA[mA], qg[mQ])]
        CB[j][mB, :] = C[np.ix_(kgB[mB], R0 + 128 * j + np.arange(128))]
    return CA, CB


# ----------------------------------------------------------------------------
# device program
# ----------------------------------------------------------------------------

def _bcast(ap_, ins_dim, n):
    """Insert a [0, n] broadcast free dim at position ins_dim of ap_.ap."""
    dims = [list(d) for d in ap_.ap]
    dims.insert(ins_dim, [0, n])
    return bass.AP(tensor=ap_.tensor, offset=ap_.offset, ap=dims)


def _restride(ap_, new_free):
    """Replace the free dims of ap_ (keep partition dim + offset)."""
    dims = [list(ap_.ap[0])] + [list(d) for d in new_free]
    return bass.AP(tensor=ap_.tensor, offset=ap_.offset, ap=dims)


def _build_program(with_bias, debug=False):
    nc = bacc.Bacc()
    E_IN = dict(kind="ExternalInput")
    qT_d = nc.dram_tensor("qT", [DM, ROWS], BF16, **E_IN)
    kT_d = nc.dram_tensor("kT", [DM, HALO], BF16, **E_IN)
    vT_d = nc.dram_tensor("vT", [DM, HALO], BF16, **E_IN)
    Wq_d = nc.dram_tensor("Wq", [DM, DM], BF16, **E_IN)
    Wk_d = nc.dram_tensor("Wk", [DM, DM], BF16, **E_IN)
    Wv_d = nc.dram_tensor("Wv", [DM, DM], BF16, **E_IN)
    Wo_d = nc.dram_tensor("Wo", [DM, DM], BF16, **E_IN)
    CA_d = nc.dram_tensor("CA", [J, 128, 136], BF16, **E_IN)
    CBs_d = nc.dram_tensor("CBs", [J, 8, 8], BF16, **E_IN)
    vsj_d = nc.dram_tensor("vsj", [1, 5 * H * 65], BF16, **E_IN)
    ident_d = nc.dram_tensor("ident", [128, 128], BF16, **E_IN)
    if with_bias:
        bq_d = nc.dram_tensor("bq", [1, DM], BF16, **E_IN)
        bk_d = nc.dram_tensor("bk", [1, DM], BF16, **E_IN)
        bv_d = nc.dram_tensor("bv", [1, DM], BF16, **E_IN)
    out_d = nc.dram_tensor("out", [ROWS, DM], BF16, kind="ExternalOutput")
    if debug:
        dbg = {nm: nc.dram_tensor(f"dbg_{nm}", [128, sz], BF16,
                                  kind="ExternalOutput")
               for nm, sz in [("qhT", 4 * ROWS), ("khT", 4 * HALO),
                              ("vh", 5 * H * 65),
                              ("eB", H * 128), ("cd", J * DM),
                              ("cT", 4 * ROWS)]}

    with tile.TileContext(nc) as tc, ExitStack() as ctx:
        sing = ctx.enter_context(tc.tile_pool(name="sing", bufs=1))
        work = ctx.enter_context(tc.tile_pool(name="work", bufs=2))
        worke = ctx.enter_context(tc.tile_pool(name="worke", bufs=4))
        psa = ctx.enter_context(tc.tile_pool(name="psa", bufs=3, space="PSUM"))
        ppn = ctx.enter_context(tc.tile_pool(name="ppn", bufs=3, space="PSUM"))

        # ---- persistent SBUF tiles -------------------------------------
        sb_vT = sing.tile([128, 4, HALO], BF16)
        sb_qT = sing.tile([128, 4, ROWS], BF16)
        sb_kT = sing.tile([128, 4, HALO], BF16)
        sb_Wv = sing.tile([128, 4, DM], BF16)
        sb_Wq = sing.tile([128, 4, DM], BF16)
        sb_Wk = sing.tile([128, 4, DM], BF16)
        sb_Wo = sing.tile([128, 4, DM], BF16)
        sb_CA = sing.tile([128, J, 136], BF16)
        sb_CBs = sing.tile([8, J, 8], BF16)

        sb_id = sing.tile([128, 128], BF16)
        sb_ones = sing.tile([1, ROWS], BF16)
        sb_eB = sing.tile([33, H, 128], BF16)
        sb_e4 = sing.tile([8, H, 8], BF16)

        sb_qhT = sing.tile([128, 4, ROWS], BF16)     # [dout(128hp+p), hp, r]
        sb_khT = sing.tile([128, 4, HALO], BF16)
        sb_vh = sing.tile([128, 5, H, 65], BF16)     # shifted row tiles
        sb_vb = sing.tile([33, 5, H, 65], BF16)      # B-halo rows + comp row@32
        sb_cd = sing.tile([128, J, DM], BF16)        # concat/attn out
        sb_cT = sing.tile([128, 4, ROWS], BF16)      # concat transposed

        # ---- DMA loads, ordered by first use ---------------------------
        # sync (SP) queue: activations; vector (DVE) queue: weights.
        nc.sync.dma_start(sb_vT[:, :, 0:264],
                          vT_d[:, 0:264].rearrange("(kc p) r -> p kc r", p=128))
        nc.scalar.dma_start(sb_Wv, Wv_d[:].rearrange("(kc p) n -> p kc n", p=128))
        nc.sync.dma_start(sb_qT, qT_d[:].rearrange("(kc p) r -> p kc r", p=128))
        nc.scalar.dma_start(sb_Wq, Wq_d[:].rearrange("(kc p) n -> p kc n", p=128))
        nc.sync.dma_start(sb_vT[:, :, 264:HALO],
                          vT_d[:, 264:HALO].rearrange("(kc p) r -> p kc r", p=128))
        nc.scalar.dma_start(sb_Wk, Wk_d[:].rearrange("(kc p) n -> p kc n", p=128))
        nc.sync.dma_start(sb_kT, kT_d[:].rearrange("(kc p) r -> p kc r", p=128))
        nc.scalar.dma_start(sb_CA, CA_d[:].rearrange("j p q -> p j q"))
        nc.scalar.dma_start(sb_Wo, Wo_d[:].rearrange("(kc p) n -> p kc n", p=128))
        nc.gpsimd.memset(sb_ones, 1.0)
        nc.gpsimd.memset(sb_eB, 0.0)
        nc.gpsimd.memset(sb_eB[32:33, :, :], 1.0)
        nc.gpsimd.memset(sb_vh[:, :, :, 64:65], 1.0)
        nc.gpsimd.dma_start(sb_CBs, CBs_d[:].rearrange("j p q -> p j q"))
        nc.gpsimd.memset(sb_vb, 0.0)
        nc.gpsimd.dma_start(sb_vb[32:33, :, :, :],
                            vsj_d[:].rearrange("p (t h d) -> p t h d", t=5, h=H))
        nc.gpsimd.dma_start(sb_id, ident_d[:])
        if with_bias:
            sb_bq = sing.tile([1, DM], BF16)
            nc.gpsimd.dma_start(sb_bq, bq_d[:])
            sb_bk = sing.tile([1, DM], BF16)
            nc.gpsimd.dma_start(sb_bk, bk_d[:])
            sb_bv = sing.tile([1, DM], BF16)
            nc.gpsimd.dma_start(sb_bv, bv_d[:])


        # evac engines round-robin
        ACTC = lambda o, i: nc.scalar.copy(o, i)
        DVEC = lambda o, i: nc.vector.tensor_copy(o, i)
        POOLC = lambda o, i: nc.gpsimd.tensor_copy(o, i)

        def vproj(rt, ps, eng):
            nr = 128 if rt < 4 else 8
            for kc in range(4):
                nc.tensor.matmul(ps[0:nr, :],
                                 sb_vT[:, kc, 128 * rt:128 * rt + nr],
                                 sb_Wv[:, kc, :], start=(kc == 0),
                                 stop=(kc == 3 and not with_bias))
            if with_bias:
                nc.tensor.matmul(ps[0:nr, :], sb_ones[0:1, 0:nr], sb_bv,
                                 start=False, stop=True)
            eng(sb_vh[0:nr, rt, :, 0:64],
                ps[0:nr, :].rearrange("p (h d) -> p h d", h=H))
            nc.gpsimd.tensor_copy(sb_vb[0:8, rt, :, :], sb_vh[0:8, rt, :, :])

        def proj_qk(hp, W, x, xcols, dst, bias, ps, eng):
            c0, c1 = xcols
            for kc in range(4):
                nc.tensor.matmul(ps[:, 0:c1 - c0],
                                 W[:, kc, 128 * hp:128 * hp + 128],
                                 x[:, kc, c0:c1], start=(kc == 0),
                                 stop=(kc == 3 and not with_bias))
            if with_bias:
                nc.tensor.matmul(ps[:, 0:c1 - c0],
                                 bias[0:1, 128 * hp:128 * hp + 128],
                                 sb_ones[0:1, 0:c1 - c0], start=False, stop=True)
            eng(dst[:, hp, c0:c1], ps[:, 0:c1 - c0])

        def scores(j, grp):
            # wide score tiles [kposA_j, q in [128j-8, 128j+128)]: the first
            # 8 q-columns are the B-piece of tile j-1 (kpos rows 0:8 there).
            hs = HE if grp == 0 else HO
            sas = []
            for t in range(2):
                sa = psa.tile([128, 2, 136], F32, tag="sa",
                              name=f"sa{j}{grp}{t}")
                if j == 0:
                    nc.vector.memset(sa[:, :, 0:8], 0.0)
                for i in range(2):
                    h = hs[2 * t + i]
                    lo = 64 * (h % 2)
                    if j == 0:
                        nc.tensor.matmul(
                            sa[:, i, 8:136],
                            sb_khT[lo:lo + 64, h // 2, 0:128],
                            sb_qhT[lo:lo + 64, h // 2, 0:128],
                            start=True, stop=True)
                    else:
                        nc.tensor.matmul(
                            sa[:, i, :],
                            sb_khT[lo:lo + 64, h // 2, 128 * j:128 * j + 128],
                            sb_qhT[lo:lo + 64, h // 2,
                                   128 * j - 8:128 * j + 128],
                            start=True, stop=True)
                sas.append(sa)
            return sas

        def psb4_all():
            # virtual tile-4 B piece for j=3 (kpos 512:520 x last 8 q),
            # both head groups; raw exp staged in sb_e4 (extracted later).
            pb = ppn.tile([8, 8, 8], F32, tag="pn", name="pb4")
            for g, hs in ((0, HE), (1, HO)):
                for i, h in enumerate(hs):
                    lo = 64 * (h % 2)
                    nc.tensor.matmul(
                        pb[0:8, 4 * g + i, :],
                        sb_khT[lo:lo + 64, h // 2, 512:520],
                        sb_qhT[lo:lo + 64, h // 2, 504:512],
                        start=True, stop=True)
            cb = sb_CBs[:, 3, :]
            nc.vector.tensor_mul(pb[0:8], pb[0:8], _bcast(cb, 1, H))
            nc.scalar.activation(sb_e4[0:8], pb[0:8], EXP)

        def psb4_ew(grp):
            ebv = sb_eB[0:8, 4 * grp:4 * grp + 4, 120:128]
            nc.gpsimd.tensor_scalar_add(
                ebv, sb_e4[0:8, 4 * grp:4 * grp + 4, :], -1.0)

        def band_ew(j, grp, sas, eA):
            """elementwise band pipeline for one head-group: C*S then exp."""
            ca = sb_CA[:, j, :]
            for t, sa in enumerate(sas):
                nc.vector.tensor_mul(sa[:], sa[:], _bcast(ca, 1, 2))
                nc.scalar.activation(eA[:, 2 * t:2 * t + 2, :], sa[:], EXP)

        def extract_eB(grp, eA_next):
            # B-piece of tile j-1 = (exp values in cols 0:8, rows 0:8 of the
            # next tile's wide eA) - 1, scattered into the zeroed eB band.
            ebv = sb_eB[0:8, 4 * grp:4 * grp + 4, 120:128]
            nc.gpsimd.tensor_scalar_add(ebv, eA_next[0:8, :, 0:8], -1.0)

        def numer(j, grp, eA):
            # psum accumulation groups must be sequential per bank: each
            # head opens (start), accumulates, and closes (stop) before the
            # next head's start, or the bank contents get reset.
            hs = HE if grp == 0 else HO
            pn = ppn.tile([128, 4, 65], F32, tag="pn", name=f"pn{j}{grp}")
            for i, h in enumerate(hs):
                nc.tensor.matmul(pn[:, i, :], eA[:, i, 8:136],
                                 sb_vh[:, j, h, :], start=True, stop=False)
                # 9-row B piece: row 8 is (eB=1) x (vb=host comp vector),
                # folding the Vcomp/denominator constant into this matmul.
                nc.tensor.matmul(pn[:, i, :], sb_eB[0:33, 4 * grp + i, :],
                                 sb_vb[0:33, j + 1, h, :],
                                 start=False, stop=True)
            return pn

        def divide(j, grp, pn):
            rf = worke.tile([128, 4], F32, tag="rf")
            nc.vector.reciprocal(rf, pn[:, :, 64])
            cd_g = _restride(sb_cd[:, j, 64 * grp:DM], [[128, 4], [1, 64]])
            nc.scalar.copy(cd_g, pn[:, :, 0:64])
            nc.gpsimd.tensor_mul(cd_g, cd_g, _bcast(rf, 2, 64))

        def transpose_rc(rc, pt):
            t = pt.tile([128, 4, 128], BF16, tag="t", name=f"t{rc}")
            for dc in range(4):
                nc.tensor.transpose(t[:, dc, :],
                                    sb_cd[:, rc, 128 * dc:128 * dc + 128],
                                    sb_id)
            (DVEC if rc % 2 == 0 else ACTC)(
                sb_cT[:, :, 128 * rc:128 * rc + 128], t)

        def oproj_rc(rc, pf):
            pfo = pf.tile([128, DM], F32, tag="f", name=f"pf{rc}")
            for dc in range(4):
                nc.tensor.matmul(pfo, sb_cT[:, dc, 128 * rc:128 * rc + 128],
                                 sb_Wo[:, dc, :], start=(dc == 0), stop=(dc == 3))
            fo = work.tile([128, DM], BF16, tag="fo")
            (ACTC if rc % 2 == 0 else DVEC)(fo, pfo)
            nc.sync.dma_start(out_d[128 * rc:128 * rc + 128, :], fo)

        # ================= phase A: projections =========================
        with tc.tile_pool(name="ppj", bufs=2, space="PSUM") as ppj:
            for rt, eng in ((0, ACTC), (1, DVEC)):
                vproj(rt, ppj.tile([128, DM], F32, tag="pj", name=f"pv{rt}"), eng)
            for hp, eng in ((0, ACTC), (1, DVEC), (2, ACTC), (3, DVEC)):
                proj_qk(hp, sb_Wq, sb_qT, (0, ROWS), sb_qhT,
                        sb_bq if with_bias else None,
                        ppj.tile([128, DM], F32, tag="pj", name=f"pq{hp}"), eng)
            for hp, eng in ((0, DVEC), (1, ACTC), (2, DVEC), (3, ACTC)):
                proj_qk(hp, sb_Wk, sb_kT, (0, 512), sb_khT,
                        sb_bk if with_bias else None,
                        ppj.tile([128, DM], F32, tag="pj", name=f"pk{hp}"), eng)
                proj_qk(hp, sb_Wk, sb_kT, (512, HALO), sb_khT,
                        sb_bk if with_bias else None,
                        ppj.tile([128, DM], F32, tag="pj", name=f"pkt{hp}"), DVEC)
            UNITS = [(jj, gg) for jj in range(J) for gg in range(2)]
            eAs = {}
            for u in (0, 1, 2):
                ju, gu = UNITS[u]
                sas = scores(ju, gu)
                eAs[u] = worke.tile([128, 4, 136], BF16, tag="eA",
                                    name=f"eA{u}")
                band_ew(ju, gu, sas, eAs[u])
            for rt, eng in ((2, DVEC), (3, ACTC)):
                vproj(rt, ppj.tile([128, DM], F32, tag="pj", name=f"pv{rt}"), eng)

        # ================= phase B: attention + output ==================
        with tc.tile_pool(name="pt", bufs=1, space="PSUM") as pt, \
             tc.tile_pool(name="pf", bufs=1, space="PSUM") as pf:
            for u in range(8):
                ju, gu = UNITS[u]
                if u + 3 < 8:
                    j3, g3 = UNITS[u + 3]
                    sas = scores(j3, g3)
                    eAs[u + 3] = worke.tile([128, 4, 136], BF16, tag="eA",
                                            name=f"eA{u + 3}")
                    band_ew(j3, g3, sas, eAs[u + 3])
                if u == 3:
                    psb4_all()
                if u <= 5:
                    extract_eB(gu, eAs[u + 2])
                else:
                    psb4_ew(gu)
                pn = numer(ju, gu, eAs.pop(u))
                divide(ju, gu, pn)
                if u == 3:
                    vproj(4, pt.tile([128, DM], F32, tag="t", name="pv4"),
                          DVEC)
                if gu == 1:
                    transpose_rc(ju, pt)
                    oproj_rc(ju, pf)
            if debug:
                nc.gpsimd.dma_start(dbg["qhT"][:],
                                    sb_qhT[:].rearrange("p a b -> p (a b)"))
                nc.gpsimd.dma_start(dbg["khT"][:],
                                    sb_khT[:].rearrange("p a b -> p (a b)"))
                nc.gpsimd.dma_start(dbg["vh"][:],
                                    sb_vh[:].rearrange("p a b c -> p (a b c)"))
                nc.gpsimd.dma_start(dbg["eB"][0:8],
                                    sb_eB[:].rearrange("p a b -> p (a b)"))
                nc.gpsimd.dma_start(dbg["cd"][:],
                                    sb_cd[:].rearrange("p a b -> p (a b)"))
                nc.gpsimd.dma_start(dbg["cT"][:],
                                    sb_cT[:].rearrange("p a b -> p (a b)"))

    if not nc.is_finalized():
        nc.finalize()
    return nc


_PROG_CACHE = {}


def _get_program(with_bias):
    import os
    dbg = bool(int(os.environ.get("BASS_KERNEL_DEBUG", "0")))
    key = (bool(with_bias), dbg)
    if key not in _PROG_CACHE:
        _PROG_CACHE[key] = _build_program(with_bias, debug=dbg)
    return _PROG_CACHE[key]


# ----------------------------------------------------------------------------
# entry point
# ----------------------------------------------------------------------------

def prep_in_maps(q, k, v, Wq, bq, Wk, bk, Wv, bv, Wo, bo, **_unused):
    """Builds per-core input maps + the traced program; returns (in_maps, nc)."""
    q = np.asarray(q, np.float32)
    k = np.asarray(k, np.float32)
    v = np.asarray(v, np.float32)
    Wq_b = np.ascontiguousarray(Wq, np.float32).astype(NP_BF16)
    Wk_b = np.ascontiguousarray(Wk, np.float32).astype(NP_BF16)
    Wv_b = np.ascontiguousarray(Wv, np.float32).astype(NP_BF16)
    Wo_b = np.ascontiguousarray(Wo, np.float32).astype(NP_BF16)
    bq = np.asarray(bq, np.float32).reshape(-1)
    bk = np.asarray(bk, np.float32).reshape(-1)
    bv = np.asarray(bv, np.float32).reshape(-1)
    with_bias = bool(np.any(bq) or np.any(bk) or np.any(bv))
    nc = _get_program(with_bias)

    C = _band_count() / np.float32(math.sqrt(DK))
    ident = np.eye(128, dtype=NP_BF16)
    Wv64 = Wv_b.astype(np.float64)
    bv64 = bv.astype(np.float64)

    in_maps = []
    for c in range(8):
        b, R0 = c // 4, ROWS * (c % 4)
        qT = np.ascontiguousarray(q[b, R0:R0 + ROWS, :].T).astype(NP_BF16)
        kT = np.zeros((DM, HALO), NP_BF16)
        vT = np.zeros((DM, HALO), NP_BF16)
        g0 = R0 - 4
        s0, s1 = max(g0, 0), min(R0 + ROWS + 4, SEQ)
        kT[:, s0 - g0:s1 - g0] = k[b, s0:s1, :].T.astype(NP_BF16)
        vT[:, s0 - g0:s1 - g0] = v[b, s0:s1, :].T.astype(NP_BF16)
        CA, CB = _c_tiles(R0, C)
        CBs = np.ascontiguousarray(CB[:, :, 120:128]).astype(NP_BF16)
        assert not CB[:, :, :120].any()
        # per-j compensation: Vcomp_j = sum_all vh - sum_{A_j valid} vh;
        # denominator constant is 2048 - 128 = 1920 exactly (every A row,
        # valid or padded, contributes +1 via the ones-column on device).
        vsum = v[b].sum(axis=0, dtype=np.float64)
        VsumW = vsum @ Wv64 + float(SEQ) * bv64
        vsj = np.zeros((5, H, 65), np.float64)
        for j in range(J):
            lo, hi = max(g0 + 128 * j, 0), min(g0 + 128 * j + 128, SEQ)
            nvalid = hi - lo
            svA = v[b, lo:hi].sum(axis=0, dtype=np.float64) @ Wv64 \
                + float(nvalid) * bv64
            vsj[j + 1, :, 0:64] = (VsumW - svA).reshape(H, DK)
            vsj[j + 1, :, 64] = float(SEQ - 128)
        m = {"qT": qT, "kT": kT, "vT": vT, "Wq": Wq_b, "Wk": Wk_b, "Wv": Wv_b,
             "Wo": Wo_b, "CA": CA.astype(NP_BF16), "CBs": CBs,
             "vsj": vsj.reshape(1, 5 * H * 65).astype(NP_BF16),
             "ident": ident}
        if with_bias:
            m["bq"] = bq[None, :].astype(NP_BF16)
            m["bk"] = bk[None, :].astype(NP_BF16)
            m["bv"] = bv[None, :].astype(NP_BF16)
        in_maps.append(m)
    return in_maps, nc


def kernel(q, k, v, Wq, bq, Wk, bk, Wv, bv, Wo, bo, **_unused):
    bo = np.asarray(bo, np.float32).reshape(-1)
    in_maps, nc = prep_in_maps(q, k, v, Wq, bq, Wk, bk, Wv, bv, Wo, bo)
    res = run_bass_kernel_spmd(nc, in_maps, core_ids=list(range(8)))
    out = np.empty((2, SEQ, DM), np.float32)
    for c in range(8):
        b, R0 = c // 4, ROWS * (c % 4)
        out[b, R0:R0 + ROWS, :] = res.results[c]["out"].astype(np.float32)
    if np.any(bo):
        out += bo
    return out


if __name__ == "__main__":
    rng = np.random.default_rng(0)
    s = 1.0 / math.sqrt(DM)
    inp = dict(
        q=rng.standard_normal((2, SEQ, DM)).astype(np.float32),
        k=rng.standard_normal((2, SEQ, DM)).astype(np.float32),
        v=rng.standard_normal((2, SEQ, DM)).astype(np.float32),
        Wq=(rng.standard_normal((DM, DM)) * s).astype(np.float32),
        bq=np.zeros(DM, np.float32),
        Wk=(rng.standard_normal((DM, DM)) * s).astype(np.float32),
        bk=np.zeros(DM, np.float32),
        Wv=(rng.standard_normal((DM, DM)) * s).astype(np.float32),
        bv=np.zeros(DM, np.float32),
        Wo=(rng.standard_normal((DM, DM)) * s).astype(np.float32),
        bo=np.zeros(DM, np.float32),
    )
    out = kernel(**inp)
    print("kernel ran, out shape", out.shape, "mean", np.abs(out).mean())
